# revision 8
# baseline (speedup 1.0000x reference)
"""Trainium2 Bass kernel for location-sensitive attention (Tacotron-style).

Data-parallel over batch B=64 across 8 NeuronCores (8 batches/core).
Math per batch b:
  pq   = tanh(query @ Wq)                         [A]
  pm   = tanh(memory @ Wm)                        [T, A]
  ploc = tanh(conv1d(awc.T) @ Wloc)               [T, A]
  e    = V . tanh(pq + ploc + pm)                 [T]
  w    = softmax(e); ctx = w @ memory             [T], [D]

Device strategy (per core, per batch):
  - memory[b] cast-loaded fp32->fp16 during DMA (SWDGE), natural [t,d] layout
  - xbar DMA transpose fp16 -> memT [d,t] (for the pm matmul, contract d)
  - conv folded into one matmul: combined = im2col_kernel @ Wloc (host-folded
    weights), im2col of awc built on-chip via overlapped-window DMA
  - pm/ploc accumulate in PSUM fp32; pq added as ACT bias; tanh on ACT
  - energies via PE matmul with V; softmax without max-subtraction
    (|e| <= sum|V| ~ 9, safe in fp32); context accumulates unnormalized
    exp(e) @ memory on PE, normalized at the end
"""
import os
import numpy as np
import ml_dtypes

import concourse.bass as bass
import concourse.bacc as bacc
import concourse.tile as tile
from concourse import mybir
from concourse.bass_utils import run_bass_kernel_spmd

F16 = mybir.dt.float16
F32 = mybir.dt.float32

B, T, QD, MD, A = 64, 2000, 1024, 512, 128
NF, KW = 32, 31
NCORE = 8
BPC = B // NCORE          # batches per core
NTF = 15                  # full 128-row t-tiles
TTAIL = T - NTF * 128     # 80
NCH, CH = 4, 500          # energy chunks along t
PADW = 2 * KW - 1 + T     # 2061 -> round up
AWCP = 2032               # padded awc length (>= T + KW - 1 = 2030)

_CACHE = {}


def _build(consts):
    """Build the per-core SPMD Bass program. consts: host-prepped numpy."""
    nc = bacc.Bacc(target_bir_lowering=False, debug=False)

    qT_d = nc.dram_tensor("qT", [QD, BPC], F32, kind="ExternalInput")
    mem_d = nc.dram_tensor("mem", [BPC * T, MD], F32, kind="ExternalInput")
    awc_d = nc.dram_tensor("awcp", [BPC * 2, AWCP], F32, kind="ExternalInput")
    octx_d = nc.dram_tensor("octx", [BPC, MD], F32, kind="ExternalOutput")
    ow_d = nc.dram_tensor("ow", [BPC, T], F32, kind="ExternalOutput")
    DBG = bool(os.environ.get("KDEBUG"))
    if DBG:
        dbg_pq = nc.dram_tensor("dbg_pq", [A, BPC], F32, kind="ExternalOutput")
        dbg_ee = nc.dram_tensor("dbg_ee", [BPC, 2048], F32, kind="ExternalOutput")
        dbg_x = nc.dram_tensor("dbg_x", [BPC, A, CH], F16, kind="ExternalOutput")
        dbg_im2 = nc.dram_tensor("dbg_im2", [BPC, 62, T], F16, kind="ExternalOutput")
        dbg_pm = nc.dram_tensor("dbg_pm", [BPC, A, CH], F16, kind="ExternalOutput")
        dbg_pl = nc.dram_tensor("dbg_pl", [BPC, A, CH], F16, kind="ExternalOutput")

    wq_i = nc.inline_tensor(consts["Wq"], name="WqI")          # [QD, A] f32
    wm_i = nc.inline_tensor(consts["Wm16"], name="WmI")        # [4,128,A] f16
    comb_i = nc.inline_tensor(consts["comb16"], name="combI")  # [62, A] f16
    v_i = nc.inline_tensor(consts["V16"], name="VI")           # [A, 1] f16
    id_i = nc.inline_tensor(consts["I16"], name="II")          # [128,128] f16

    with tile.TileContext(nc) as tc:
        with (
            tc.tile_pool(name="consts", bufs=1) as cpool,
            tc.tile_pool(name="big", bufs=2) as big,
            tc.tile_pool(name="small", bufs=2) as sm,
            tc.tile_pool(name="chunk", bufs=3) as chp,
            tc.tile_pool(name="ps_pm", bufs=2, space="PSUM") as ps_pm,
            tc.tile_pool(name="ps_pl", bufs=2, space="PSUM") as ps_pl,
            tc.tile_pool(name="ps_e", bufs=2, space="PSUM") as ps_e,
            tc.tile_pool(name="ps_c", bufs=2, space="PSUM") as ps_c,
        ):
            # ---- constants into SBUF ----
            wm16 = cpool.tile([128, 4, A], F16)
            nc.sync.dma_start(out=wm16, in_=wq_rearr(wm_i))
            comb16 = cpool.tile([62, A], F16)
            nc.sync.dma_start(out=comb16, in_=comb_i[:, :])
            v16 = cpool.tile([A, 1], F16)
            nc.sync.dma_start(out=v16, in_=v_i[:, :])
            i16 = cpool.tile([128, 128], F16)
            nc.sync.dma_start(out=i16, in_=id_i[:, :])
            wq32_raw = cpool.tile([128, 8, A], F32)
            nc.sync.dma_start(
                out=wq32_raw, in_=wq_i[:, :].rearrange("(k p) a -> p k a", p=128)
            )
            qt32_raw = cpool.tile([128, 8, BPC], F32)
            nc.sync.dma_start(
                out=qt32_raw, in_=qT_d[:, :].rearrange("(k p) b -> p k b", p=128)
            )
            # stage via DVE so the fp32 matmul has a single-producer wait
            wq32 = cpool.tile([128, 8, A], F32)
            nc.vector.tensor_copy(out=wq32, in_=wq32_raw)
            qt32 = cpool.tile([128, 8, BPC], F32)
            nc.vector.tensor_copy(out=qt32, in_=qt32_raw)

            # ---- pq = tanh(query @ Wq), laid [A, BPC] ----
            pq_ps = ps_c.tile([A, BPC], F32, tag="ctx")
            for k in range(8):
                nc.tensor.matmul(
                    pq_ps, lhsT=wq32[:, k, :], rhs=qt32[:, k, :],
                    start=(k == 0), stop=(k == 7),
                )
            pqt32 = cpool.tile([A, BPC], F32)
            nc.scalar.activation(
                out=pqt32, in_=pq_ps, func=mybir.ActivationFunctionType.Tanh
            )
            if DBG:
                nc.sync.dma_start(out=dbg_pq[:, :], in_=pqt32)

            for ib in range(BPC):
                r0 = ib * T
                # ---- memory cast-load fp32 -> fp16, natural [t, d] ----
                mem16 = big.tile([128, NTF + 1, MD], F16, tag="mem16")
                nc.gpsimd.dma_start(
                    out=mem16[:, 0:NTF, :],
                    in_=mem_d[r0 : r0 + NTF * 128, :].rearrange(
                        "(i p) d -> p i d", p=128
                    ),
                )
                nc.gpsimd.dma_start(
                    out=mem16[0:TTAIL, NTF, :],
                    in_=mem_d[r0 + NTF * 128 : r0 + T, :],
                )
                # ---- xbar transpose -> memT [d, t] as [128, 4, T] ----
                memt16 = big.tile([128, 4, T], F16, tag="memt16")
                for i in range(NTF):
                    nc.sync.dma_start_transpose(
                        out=memt16[:, :, i * 128 : (i + 1) * 128],
                        in_=mem16[:, i, :],
                    )
                nc.sync.dma_start_transpose(
                    out=memt16[:, :, NTF * 128 : T],
                    in_=mem16[0:TTAIL, NTF, :],
                )
                # ---- awc pad + im2col (overlapped windows) ----
                awc16 = sm.tile([2, AWCP], F16, tag="awc")
                nc.gpsimd.dma_start(
                    out=awc16, in_=awc_d[ib * 2 : ib * 2 + 2, :]
                )
                im2 = sm.tile([62, T], F16, tag="im2")
                a_ap = awc16[:, :]
                src = bass.AP(
                    tensor=a_ap.tensor,
                    offset=a_ap.offset,
                    # partition step = tile row size (flat element space)
                    ap=[[AWCP, 2], [1, KW], [1, T]],
                )
                nc.gpsimd.dma_start(out=im2, in_=src)

                eexp32 = sm.tile([1, 2048], F32, tag="eexp")
                for j in range(NCH):
                    sl = slice(j * CH, (j + 1) * CH)
                    # ploc chunk
                    ploc_ps = ps_pl.tile([A, CH], F32, tag="ploc")
                    nc.tensor.matmul(
                        ploc_ps, lhsT=comb16, rhs=im2[:, sl], start=True, stop=True
                    )
                    ploct16 = chp.tile([A, CH], F16, tag="ploc16")
                    nc.scalar.activation(
                        out=ploct16, in_=ploc_ps,
                        func=mybir.ActivationFunctionType.Tanh,
                    )
                    # pm chunk (reference tanh's pm before the outer sum)
                    pm_ps = ps_pm.tile([A, CH], F32, tag="pm")
                    for m in range(4):
                        nc.tensor.matmul(
                            pm_ps, lhsT=wm16[:, m, :], rhs=memt16[:, m, sl],
                            start=(m == 0), stop=(m == 3),
                        )
                    pm16 = chp.tile([A, CH], F16, tag="pm16")
                    nc.scalar.activation(
                        out=pm16, in_=pm_ps,
                        func=mybir.ActivationFunctionType.Tanh,
                    )
                    if DBG and j == 0:
                        nc.sync.dma_start(out=dbg_pm[ib, :, :], in_=pm16)
                        nc.sync.dma_start(out=dbg_pl[ib, :, :], in_=ploct16)
                    xs16 = chp.tile([A, CH], F16, tag="xs16")
                    nc.vector.tensor_add(xs16, pm16, ploct16)
                    x16 = chp.tile([A, CH], F16, tag="x16")
                    nc.scalar.activation(
                        out=x16, in_=xs16,
                        func=mybir.ActivationFunctionType.Tanh,
                        bias=pqt32[:, ib : ib + 1],
                    )
                    if DBG and j == 0:
                        nc.sync.dma_start(out=dbg_x[ib, :, :], in_=x16)
                    e_ps = ps_e.tile([1, CH], F32, tag="e")
                    nc.tensor.matmul(e_ps, lhsT=v16, rhs=x16, start=True, stop=True)
                    nc.scalar.activation(
                        out=eexp32[:, sl], in_=e_ps,
                        func=mybir.ActivationFunctionType.Exp,
                    )
                if DBG:
                    nc.sync.dma_start(out=dbg_ee[ib : ib + 1, :], in_=eexp32)
                    nc.sync.dma_start(out=dbg_im2[ib, :, :], in_=im2)
                # ---- softmax (no max-subtraction) ----
                s32 = sm.tile([1, 1], F32, tag="s32")
                nc.vector.reduce_sum(
                    out=s32, in_=eexp32[:, 0:T], axis=mybir.AxisListType.X
                )
                r32 = sm.tile([1, 1], F32, tag="r32")
                nc.vector.reciprocal(out=r32, in_=s32)
                w32 = sm.tile([1, T], F32, tag="w32")
                nc.vector.tensor_scalar_mul(out=w32, in0=eexp32[:, 0:T], scalar1=r32)
                nc.sync.dma_start(out=ow_d[ib : ib + 1, :], in_=w32)
                # unnormalized weights, transposed to [t(p), i] for context.
                # SBUF->SBUF partition scatter is not expressible in one DMA,
                # so bounce through a small DRAM scratch (cast to fp16 on the
                # way back).
                scr = nc.dram_tensor(f"scr{ib}", [2048], F32, kind="Internal")
                w_inst = nc.sync.dma_start(out=scr[:], in_=eexp32[:, :])
                wt16 = sm.tile([128, 16], F16, tag="wt16")
                r_inst = nc.gpsimd.dma_start(
                    out=wt16, in_=scr[:].rearrange("(i p) -> p i", p=128)
                )
                tile.add_dep_helper(
                    r_inst.ins, w_inst.ins, reason="dram scratch RAW"
                )
                # ---- context = (exp(e) @ mem) * (1/sum) ----
                ctx_ps = ps_c.tile([1, MD], F32, tag="ctx")
                for i in range(NTF):
                    nc.tensor.matmul(
                        ctx_ps, lhsT=wt16[:, i : i + 1], rhs=mem16[:, i, :],
                        start=(i == 0), stop=False,
                    )
                nc.tensor.matmul(
                    ctx_ps, lhsT=wt16[0:TTAIL, NTF : NTF + 1],
                    rhs=mem16[0:TTAIL, NTF, :], start=False, stop=True,
                )
                ctx32 = sm.tile([1, MD], F32, tag="ctx32")
                nc.vector.tensor_scalar_mul(out=ctx32, in0=ctx_ps, scalar1=r32)
                nc.sync.dma_start(out=octx_d[ib : ib + 1, :], in_=ctx32)

    nc.compile()
    return nc


def wq_rearr(wm_i):
    # WmI stored [4, 128, A]; SBUF wants [p, m, a]
    return wm_i[:, :, :].rearrange("m p a -> p m a")


def _prep_consts(Wq, Wm, conv_kernel, Wloc, V):
    Wm16 = np.ascontiguousarray(
        Wm.reshape(4, 128, A).astype(np.float16)
    )
    # combined[r, a] for r = c*31 + k: sum_f conv_kernel[k, c, f] * Wloc[f, a]
    km = np.transpose(conv_kernel, (1, 0, 2)).reshape(62, NF)  # [c*31+k, f]
    comb = (km.astype(np.float64) @ Wloc.astype(np.float64)).astype(np.float32)
    comb16 = np.ascontiguousarray(comb.astype(np.float16))
    V16 = np.ascontiguousarray(V.reshape(A, 1).astype(np.float16))
    I16 = np.ascontiguousarray(np.eye(128, dtype=np.float16))
    return {
        "Wq": np.ascontiguousarray(Wq.astype(np.float32)),
        "Wm16": Wm16,
        "comb16": comb16,
        "V16": V16,
        "I16": I16,
    }


def kernel(query, memory, attention_weights_cat, Wq, Wm, conv_kernel, Wloc, V):
    query = np.asarray(query, dtype=np.float32)
    memory = np.asarray(memory, dtype=np.float32)
    awc = np.asarray(attention_weights_cat, dtype=np.float32)

    key = "prog"
    if key not in _CACHE:
        consts = _prep_consts(
            np.asarray(Wq, np.float32), np.asarray(Wm, np.float32),
            np.asarray(conv_kernel, np.float32), np.asarray(Wloc, np.float32),
            np.asarray(V, np.float32),
        )
        _CACHE[key] = _build(consts)
    nc = _CACHE[key]

    # host prep: transpose query, pad awc
    awcp = np.zeros((B, 2, AWCP), dtype=np.float32)
    awcp[:, :, KW // 2 : KW // 2 + T] = awc
    in_maps = []
    for c in range(NCORE):
        bs = slice(c * BPC, (c + 1) * BPC)
        in_maps.append({
            "qT": np.ascontiguousarray(query[bs].T),
            "mem": np.ascontiguousarray(memory[bs].reshape(BPC * T, MD)),
            "awcp": np.ascontiguousarray(awcp[bs].reshape(BPC * 2, AWCP)),
        })
    res = run_bass_kernel_spmd(
        nc, in_maps, core_ids=list(range(NCORE)),
        trace=bool(os.environ.get("KBENCH_TRACE")),
    )
    if res.exec_time_ns is not None:
        print(f"HW exec time: {res.exec_time_ns} ns")
    ctx = np.concatenate([r["octx"] for r in res.results], axis=0)
    aw = np.concatenate([r["ow"] for r in res.results], axis=0)
    return ctx, aw
